# revision 3
# baseline (speedup 1.0000x reference)
"""Trainium2 Bass kernel for Bahdanau additive cross-attention + softmax +
weighted sum + residual + LayerNorm — separable sine-series formulation.

tanh(u) ~= sum_{k=1..K} b_k sin(k*pi*u/P)  (P=11.5, K=16, weighted LS,
~1e-3 in the data-dense region), and sin(k(tx+tc)) splits by the angle
addition formula, so

    scores = sum_k [ (b_k s_d sin(k tx))^T cos(k tc)
                   + (b_k s_d cos(k tx))^T sin(k tc) ]

i.e. 4K rank-128 accumulating PE matmuls; the 16.7M-element tanh is gone.
Per-engine split:
  - ACT: base sin/cos(theta) per side (args within +-pi), plus sin(2θ),
    softmax Exp, and the PSUM→SBUF wT copies.
  - DVE: harmonics k=3..K via double-step Chebyshev Z_k = D2*Z_{k-2} -
    Z_{k-4} on PAIRED [128,2,N] fp16 tiles (sin|cos share one op via a
    stride-0 broadcast of D2, 2x perf class), plus the b_k*s_d scalings
    as single 4x-mode tensor_scalar ops per k.
  - PE: 4K fp16 matmuls, emitted half-major so the first softmax starts
    while the second half's matmuls still run.
Softmax / weighted-sum / LayerNorm epilogue unchanged.
Sharding: data-parallel over batch B=8, one batch element per NeuronCore.
"""

import numpy as np
from contextlib import ExitStack

import concourse.bass as bass
import concourse.bacc as bacc
import concourse.tile as tile
from concourse import mybir
from concourse.masks import make_identity
from concourse.bass_utils import run_bass_kernel_spmd

TQ, TV, D, B = 256, 512, 128, 8
N_CORES = 8
LN_EPS = 1e-3
F32 = mybir.dt.float32
F16 = mybir.dt.float16

import os as _os

P_FIT = 11.5
K_FIT = int(_os.environ.get("CROSSATTN_K", "16"))
# how many k's L-scaling (bs_k * Zx_k) runs on ACT instead of DVE
ACT_SCALE_K = int(_os.environ.get("CROSSATTN_ACT_SCALE_K", "16"))
# weighted-LS sine coefficients for tanh on [-10,10], P=11.5, K=16
B_COEF = [
    1.2427899585495437,
    -0.013808549060592812,
    0.3433439205483932,
    -0.01681123428596712,
    0.1438706165356765,
    -0.010263818301363725,
    0.06025372513364109,
    -0.0018911516065990622,
    0.02147140243061826,
    0.00402361066818695,
    0.004401459283791823,
    0.005514870716163117,
    -0.0010537311765682757,
    0.004904021231167683,
    -0.00257684886987355,
    0.002737580295355473,
]


def _body(ctx, tc, x_d, c_d, s_d, g_d, b_d, o_d, repeats=1, loop_iters=1):
    nc = tc.nc
    AF = mybir.ActivationFunctionType

    singles = ctx.enter_context(tc.tile_pool(name="singles", bufs=1))
    trig_pool = ctx.enter_context(tc.tile_pool(name="trig", bufs=2))
    lhs_pool = ctx.enter_context(tc.tile_pool(name="lhs", bufs=2))
    w_pool = ctx.enter_context(tc.tile_pool(name="w", bufs=2))
    vec_pool = ctx.enter_context(tc.tile_pool(name="vec", bufs=4))
    y_pool = ctx.enter_context(tc.tile_pool(name="y", bufs=2))
    out_pool = ctx.enter_context(tc.tile_pool(name="o", bufs=2))
    ps_scores = ctx.enter_context(tc.tile_pool(name="ps_s", bufs=2, space="PSUM"))
    ps_tr = ctx.enter_context(tc.tile_pool(name="ps_t", bufs=2, space="PSUM"))
    ps_attn = ctx.enter_context(tc.tile_pool(name="ps_a", bufs=2, space="PSUM"))

    ident = singles.tile([128, 128], F32)
    make_identity(nc, ident)

    # x rows in natural layout [i(part), t, d] — residual input
    xsb = singles.tile([128, 2, D], F32)
    for t in range(2):
        nc.sync.dma_start(xsb[:, t, :], x_d[t * 128:(t + 1) * 128, :])
    # context rows in natural layout [j(part), jc, d] — attn matmul rhs
    csb = singles.tile([128, 4, D], F32)
    for jc in range(4):
        nc.sync.dma_start(csb[:, jc, :], c_d[jc * 128:(jc + 1) * 128, :])

    # transposed copies: xT[d, i], cT[d, j] (fp32)
    xT = singles.tile([128, TQ], F32)
    for t in range(2):
        pt = ps_tr.tile([128, 128], F32)
        nc.tensor.transpose(pt, xsb[:, t, :], ident)
        nc.vector.tensor_copy(xT[:, t * 128:(t + 1) * 128], pt)
    cT = singles.tile([128, TV], F32)
    for jc in range(4):
        pt = ps_tr.tile([128, 128], F32)
        nc.tensor.transpose(pt, csb[:, jc, :], ident)
        nc.vector.tensor_copy(cT[:, jc * 128:(jc + 1) * 128], pt)

    ident16 = singles.tile([128, 128], F16)
    nc.gpsimd.tensor_copy(ident16, ident)
    csb16 = singles.tile([128, 4, D], F16)
    nc.gpsimd.tensor_copy(csb16, csb)

    # constant bias column for the cos-via-sin(θ+π/2) trick
    half_pi = singles.tile([128, 1], F32)
    nc.vector.memset(half_pi, float(np.pi / 2))

    # per-k lhsT scale columns: bs[:, k] = scale * b_k
    scale_col = singles.tile([128, 1], F32)
    nc.sync.dma_start(scale_col, bass.AP(s_d, 0, [[1, 128], [1, 1]]))
    bs = singles.tile([128, K_FIT], F32)
    for k in range(K_FIT):
        nc.vector.tensor_scalar(bs[:, k:k + 1], scale_col, float(B_COEF[k]),
                                None, op0=mybir.AluOpType.mult)

    # gamma/beta broadcast across partitions
    gamma_b = singles.tile([128, D], F32)
    nc.gpsimd.dma_start(gamma_b, bass.AP(g_d, 0, [[0, 128], [1, 128]]))
    beta_b = singles.tile([128, D], F32)
    nc.gpsimd.dma_start(beta_b, bass.AP(b_d, 0, [[0, 128], [1, 128]]))

    if loop_iters > 1:
        env = locals()
        with tc.For_i(0, loop_iters, 1,
                      hint_engines=(mybir.EngineType.PE, mybir.EngineType.DVE)):
            _main_pass(tc, ctx, env)
    else:
        for _rep in range(repeats):
            _main_pass(tc, ctx, locals())


def _bcast2(ap_tile):
    """[128, N] tile AP -> [128, 2, N] with a stride-0 middle dim."""
    return bass.AP(ap_tile.tensor, ap_tile.offset,
                   [list(ap_tile.ap[0]), [0, 2], list(ap_tile.ap[1])])


def _trig_phase(nc, trig_pool, lhs_pool, xT, cT, bs, half_pi):
    """Build paired harmonic tiles Z_k = [sin(kθ) | cos(kθ)] per side and
    the scaled x-side lhsT pairs; return (L, Zc) dicts."""
    AF = mybir.ActivationFunctionType
    ALU = mybir.AluOpType
    F = F16
    w0 = float(np.pi / P_FIT)
    K = K_FIT

    Zx = {}
    Zc = {}
    D2map = {}
    L = {}

    def scale_L(k):
        L[k] = lhs_pool.tile([128, 2, TQ], F, tag=f"L{k}", name=f"L{k}")
        if k > K - ACT_SCALE_K:
            nc.scalar.mul(L[k], Zx[k], bs[:, k - 1:k])
        else:
            nc.vector.tensor_scalar(L[k], Zx[k], bs[:, k - 1:k], None,
                                    op0=ALU.mult)

    # k = 1, 2 bases + D2 per side, interleaved c-then-x so matmul k can
    # fire as soon as both sides of k exist.
    for side, th, N, Z in (("c", cT, TV, Zc), ("x", xT, TQ, Zx)):
        Z[1] = trig_pool.tile([128, 2, N], F, tag=f"Z1{side}", name=f"Z1{side}")
        nc.scalar.activation(Z[1][:, 0, :], th, AF.Sin, scale=w0)
        nc.scalar.activation(Z[1][:, 1, :], th, AF.Sin, scale=w0, bias=half_pi)
    scale_L(1)
    for side, Z in (("c", Zc), ("x", Zx)):
        th = cT if side == "c" else xT
        N = TV if side == "c" else TQ
        Z[2] = trig_pool.tile([128, 2, N], F, tag=f"Z2{side}", name=f"Z2{side}")
        # sin(2θ) directly on ACT (|2θ| < π); cos(2θ) = 2cos²θ-1 on DVE
        nc.scalar.activation(Z[2][:, 0, :], th, AF.Sin, scale=2 * w0)
        nc.vector.tensor_tensor(Z[2][:, 1, :], Z[1][:, 1, :], Z[1][:, 1, :],
                                op=ALU.mult)
        nc.vector.tensor_scalar(Z[2][:, 1, :], Z[2][:, 1, :], 2.0, -1.0,
                                op0=ALU.mult, op1=ALU.add)
        D2 = trig_pool.tile([128, N], F, tag=f"D2{side}", name=f"D2{side}")
        nc.vector.tensor_scalar(D2, Z[2][:, 1, :], 2.0, None, op0=ALU.mult)
        D2map[side] = _bcast2(D2)
    scale_L(2)

    # harmonics k >= 3, c side first within each k
    for k in range(3, K + 1):
        for side, Z in (("c", Zc), ("x", Zx)):
            N = TV if side == "c" else TQ
            D2b = D2map[side]
            Z[k] = trig_pool.tile([128, 2, N], F, tag=f"Z{k}{side}",
                                  name=f"Z{k}{side}")
            if k == 3:
                # S3 = D2·S1 + S1 ; C3 = D2·C1 − C1  (Z[-1] = [−S1 | C1])
                nc.vector.tensor_tensor(Z[3], D2b, Z[1], op=ALU.mult)
                nc.vector.tensor_tensor(Z[3][:, 0, :], Z[3][:, 0, :],
                                        Z[1][:, 0, :], op=ALU.add)
                nc.vector.tensor_tensor(Z[3][:, 1, :], Z[3][:, 1, :],
                                        Z[1][:, 1, :], op=ALU.subtract)
            elif k == 4:
                # S4 = D2·S2 ; C4 = D2·C2 − 1
                nc.vector.tensor_tensor(Z[4], D2b, Z[2], op=ALU.mult)
                nc.vector.tensor_scalar(Z[4][:, 1, :], Z[4][:, 1, :], 1.0,
                                        -1.0, op0=ALU.mult, op1=ALU.add)
            else:
                nc.vector.tensor_tensor(Z[k], D2b, Z[k - 2], op=ALU.mult)
                nc.vector.tensor_tensor(Z[k], Z[k], Z[k - 4], op=ALU.subtract)
        scale_L(k)
    return L, Zc


def _main_pass(tc, ctx, env):
    nc = tc.nc
    AF = mybir.ActivationFunctionType
    ALU = mybir.AluOpType
    (trig_pool, lhs_pool, w_pool, vec_pool, y_pool, out_pool, ps_scores,
     ps_tr, ps_attn, ident16, xsb, csb16, xT, cT, bs, half_pi, gamma_b,
     beta_b, o_d) = (
        env["trig_pool"], env["lhs_pool"], env["w_pool"], env["vec_pool"],
        env["y_pool"], env["out_pool"], env["ps_scores"], env["ps_tr"],
        env["ps_attn"], env["ident16"], env["xsb"], env["csb16"], env["xT"],
        env["cT"], env["bs"], env["half_pi"], env["gamma_b"], env["beta_b"],
        env["o_d"])

    L, Zc = _trig_phase(nc, trig_pool, lhs_pool, xT, cT, bs, half_pi)
    K = K_FIT

    # half-major accumulation: half 0's chain completes first so its
    # softmax overlaps half 1's matmuls.
    scores = [ps_scores.tile([128, TV], F32, tag=f"sc{h}", name=f"sc{h}")
              for h in range(2)]
    for h in range(2):
        sl = slice(h * 128, (h + 1) * 128)
        for k in range(1, K + 1):
            nc.tensor.matmul(scores[h], L[k][:, 0, sl], Zc[k][:, 1, :],
                             start=(k == 1), stop=False)
            nc.tensor.matmul(scores[h], L[k][:, 1, sl], Zc[k][:, 0, :],
                             start=False, stop=(k == K))

    for t in range(2):
        neg_max = vec_pool.tile([128, 1], F32)
        nc.vector.reduce_max(neg_max, scores[t], axis=mybir.AxisListType.X,
                             negate=True)
        w = w_pool.tile([128, TV], F16)
        sum_exp = vec_pool.tile([128, 1], F32)
        nc.scalar.activation(w, scores[t], AF.Exp, bias=neg_max,
                             accum_out=sum_exp)
        r = vec_pool.tile([128, 1], F32)
        nc.vector.reciprocal(r, sum_exp)

        wT = w_pool.tile([128, 4, 128], F16, tag="wT")
        for jc in range(4):
            pt = ps_tr.tile([128, 128], F16)
            nc.tensor.transpose(pt, w[:, jc * 128:(jc + 1) * 128], ident16)
            nc.scalar.copy(wT[:, jc, :], pt)
        attn = ps_attn.tile([128, D], F32)
        for jc in range(4):
            nc.tensor.matmul(attn, wT[:, jc, :], csb16[:, jc, :],
                             start=(jc == 0), stop=(jc == 3))

        # y = attn * (1/sum_exp) + x
        y = y_pool.tile([128, D], F32)
        nc.vector.scalar_tensor_tensor(y, in0=attn, scalar=r,
                                       in1=xsb[:, t, :],
                                       op0=ALU.mult, op1=ALU.add)
        stats = vec_pool.tile([128, 6], F32)
        nc.vector.bn_stats(stats, y)
        mv = vec_pool.tile([128, 2], F32)
        nc.vector.bn_aggr(mv, stats)

        # rstd = rsqrt(var + eps) on DVE: Quake seed + 3 Newton steps.
        v = vec_pool.tile([128, 1], F32)
        nc.vector.tensor_scalar_add(v, mv[:, 1:2], LN_EPS)
        yi = vec_pool.tile([128, 1], mybir.dt.int32)
        nc.vector.tensor_scalar(yi, v.bitcast(mybir.dt.int32), 1, None,
                                op0=ALU.arith_shift_right)
        nc.vector.tensor_scalar(yi, yi, -1, 0x5F3759DF,
                                op0=ALU.mult, op1=ALU.add)
        rs = yi.bitcast(F32)
        u = vec_pool.tile([128, 1], F32)
        h2 = vec_pool.tile([128, 1], F32)
        for _ in range(2):
            nc.vector.tensor_scalar(u, rs, rs, v, op0=ALU.mult, op1=ALU.mult)
            nc.vector.tensor_scalar(h2, u, -0.5, 1.5, op0=ALU.mult, op1=ALU.add)
            nc.vector.tensor_scalar(rs, rs, h2, None, op0=ALU.mult)

        t1 = out_pool.tile([128, D], F32)
        nc.vector.tensor_scalar(t1, y, mv[:, 0:1], rs,
                                op0=ALU.subtract, op1=ALU.mult)
        t2 = out_pool.tile([128, D], F32)
        nc.vector.tensor_mul(t2, t1, gamma_b)
        t3 = out_pool.tile([128, D], F32)
        nc.vector.tensor_add(t3, t2, beta_b)
        nc.sync.dma_start(o_d[t * 128:(t + 1) * 128, :], t3)


def build_nc(repeats=1, loop_iters=1):
    nc = bacc.Bacc("TRN2", target_bir_lowering=False)
    x_d = nc.dram_tensor("x", [TQ, D], F32, kind="ExternalInput")
    c_d = nc.dram_tensor("context", [TV, D], F32, kind="ExternalInput")
    s_d = nc.dram_tensor("scale", [D], F32, kind="ExternalInput")
    g_d = nc.dram_tensor("gamma", [D], F32, kind="ExternalInput")
    b_d = nc.dram_tensor("beta", [D], F32, kind="ExternalInput")
    o_d = nc.dram_tensor("out", [TQ, D], F32, kind="ExternalOutput")
    with tile.TileContext(nc) as tc:
        with ExitStack() as ctx:
            _body(ctx, tc, x_d, c_d, s_d, g_d, b_d, o_d, repeats=repeats,
                  loop_iters=loop_iters)
    nc.compile()
    return nc


_NC_CACHE = None


def _get_nc():
    global _NC_CACHE
    if _NC_CACHE is None:
        _NC_CACHE = build_nc()
    return _NC_CACHE


def kernel(**inputs) -> np.ndarray:
    x = np.ascontiguousarray(np.asarray(inputs["x"], dtype=np.float32))
    context = np.ascontiguousarray(np.asarray(inputs["context"], dtype=np.float32))
    scale = np.ascontiguousarray(np.asarray(inputs["scale"], dtype=np.float32))
    gamma = np.ascontiguousarray(np.asarray(inputs["gamma"], dtype=np.float32))
    beta = np.ascontiguousarray(np.asarray(inputs["beta"], dtype=np.float32))

    nc = _get_nc()
    in_maps = [
        {
            "x": x[b],
            "context": context[b],
            "scale": scale,
            "gamma": gamma,
            "beta": beta,
        }
        for b in range(B)
    ]
    res = run_bass_kernel_spmd(nc, in_maps, core_ids=list(range(N_CORES)))
    return np.stack([res.results[b]["out"] for b in range(B)], axis=0)
